# revision 9
# baseline (speedup 1.0000x reference)
"""Trainium2 Bass kernel for segment_sum (GAT reduce-sum stage).

out[n, :] = sum over edges e with dst[e] == n of msg[e, :],  n in [0, 50000).

Strategy v3 (8 NeuronCores, SPMD single program): identity-stationary slot
accumulation.

  - Host sorts nodes by degree and packs 128 consecutive (sorted) nodes per
    tile; a "position" is 8 consecutive tiles, one per core, so all cores run
    an identical schedule s[i] = max degree within position i. Each node's
    edges fill slots w = 0..deg-1 of its partition row (zero-padded to s[i]).
    Because degrees are dense small integers, sorting makes the padding ~1%
    (vs ~24% for the per-node class scheme this replaces).
  - msg ships as fp8 e3m4 quantized with per-node error feedback (the
    quantization residual of slot w is carried into slot w+1), so the
    on-device fp32 sum of the quantized slots tracks the exact per-node sum;
    measured rel err ~4e-3 vs the 2e-2 budget. DMA traffic is ~10 MB/core.
  - Device: the PE stationary operand is a [128, 128] identity, loaded from
    the first matmul and identical for every matmul in the program. Each
    matmul accumulates one [128 nodes, 64 feat] slot chunk into PSUM:
    out[m, n] += sum_p I[p, m] * chunk[p, n] = chunk[m, n]. The moving side
    streams 64 columns (~27 ns/chunk); there is no per-tile weight churn,
    no on-device one-hot build, and DVE stays idle.
  - Output tiles accumulate into a persistent SBUF strip; one large store
    per pass. No collectives: cores own disjoint node sets; host scatters.
"""

import numpy as np
import ml_dtypes

import concourse.tile as tile
from concourse import bass, mybir
from concourse.bass_utils import run_bass_kernel_spmd
from concourse.vector_clock import ScopedClock

P = 128          # partitions / tile node count
F = 64           # feature dim
N_CORES = 8
NUM_NODES = 50000
NPOS = -(-NUM_NODES // (P * N_CORES))          # 49 positions per core
NODE_SLOTS = NPOS * P * N_CORES                # 50176 (176 dummy slots)

_FP8 = mybir.dt.float8e3                       # e3m4
_NP8 = ml_dtypes.float8_e3m4
_FP32 = mybir.dt.float32

_DMA_BATCH_CHUNKS = 256    # 256 * 64 B = 16 KiB / partition per load (~2 MB)

_MAX_INST_WAITS = 1


def _split_excess_waits(nc, max_waits: int = _MAX_INST_WAITS):
    """This walrus build rejects instructions carrying more than `max_waits`
    sem waits ("Too many sync wait commands"), but Tile's wait pass piles
    every needed proc wait onto the consuming instruction. Hoist the excess
    onto wait-only EventSemaphore instructions inserted just before, on the
    same engine queue (same semantics: queue is in-order)."""
    n = 0
    for f in nc.m.functions:
        for b in f.blocks:
            il = b.instructions
            out = []
            changed = False
            for inst in il:
                si = inst.sync_info
                if si is not None and si.on_wait and len(si.on_wait) > max_waits:
                    waits = list(si.on_wait)
                    extra, keep = waits[:-max_waits], waits[-max_waits:]
                    for i in range(0, len(extra), max_waits):
                        ev = mybir.InstEventSemaphore(
                            name=f"{inst.name}-wsplit{n}",
                            engine=inst.engine,
                            ins=[],
                            outs=[],
                            sync_info=mybir.SyncInfo(
                                on_wait=extra[i:i + max_waits], on_update=[]),
                        )
                        n += 1
                        out.append(ev)
                    inst.sync_info = mybir.SyncInfo(
                        on_wait=keep, on_update=list(si.on_update))
                    changed = True
                out.append(inst)
            if changed:
                b.instructions = out


def _patched_drain_and_barrier(self, tick_clock, wait_clock):
    nc = self.nc
    probe = nc.sync.nop(nofuse=True, hint="drain_waits")
    wait_clock.add_sem_waits(probe.ins, ScopedClock({None: tick_clock.global_clock}))
    si = probe.ins.sync_info
    waits = list(si.on_wait) if si is not None else []
    if si is not None:
        del si.on_wait[:]
    by_name = {h.name: h for h in self.sems.allocated().values()}
    for w in waits:
        assert w.wait_reg is None
        nc.sync.wait_ge(by_name[w.ant_name], w.wait_value)
    nc.sync.drain()

    nc.all_engine_barrier()
    popped = nc._tile_sem_poison_stack.pop()
    assert popped is self._sem_poison
    nc.clear_and_free_semaphores(list(self.sems.allocated().values()))
    nc.all_engine_barrier()

    _split_excess_waits(nc)


tile.TileContext._drain_and_barrier = _patched_drain_and_barrier


def build_program_v3(schedule, n_cores: int = N_CORES,
                     repeat: int = 1) -> bass.Bass:
    """schedule: per-position slot counts s[i] (same for every core).

    Inputs per core: msg [P, TOT*F] fp8 (TOT = sum(s); position i occupies
    chunk columns off[i]..off[i]+s[i], chunk w = slot-w edge of the
    partition's node), ident [P, P] fp8. Output: [P, NPOS*F] fp32
    (position-major strip; host transposes/scatters).

    repeat > 1 re-runs the whole body (for steady-state timing via the
    T(N) slope; each repeat writes the same output)."""
    npos = len(schedule)
    tot = int(sum(schedule))
    nc = bass.Bass("TRN2", target_bir_lowering=False, debug=False,
                   num_devices=n_cores)
    msg_d = nc.dram_tensor("msg", [P, tot * F], _FP8,
                           kind="ExternalInput").ap()
    id_d = nc.dram_tensor("ident", [P, P], _FP8, kind="ExternalInput").ap()
    out_d = nc.dram_tensor("out", [P, npos * F], _FP32,
                           kind="ExternalOutput").ap()

    # batch positions into DMA loads of <= _DMA_BATCH_CHUNKS chunks
    batches = []          # (chunk_off, nchunks, [(pos, local_chunk_off), ...])
    cur_off, cur_n, cur_pos = 0, 0, []
    off = 0
    for i, s in enumerate(schedule):
        if cur_n + s > _DMA_BATCH_CHUNKS and cur_n > 0:
            batches.append((cur_off, cur_n, cur_pos))
            cur_off, cur_n, cur_pos = off, 0, []
        cur_pos.append((i, cur_n))
        cur_n += s
        off += s
    if cur_n:
        batches.append((cur_off, cur_n, cur_pos))

    with tile.TileContext(nc) as tc:
        with (
            tc.tile_pool(name="const", bufs=1) as cpool,
            tc.tile_pool(name="msg", bufs=3) as mpool,
            tc.tile_pool(name="psum", bufs=8, space="PSUM") as ppool,
        ):
            id_t = cpool.tile([P, P], _FP8)
            nc.sync.dma_start(out=id_t[:], in_=id_d[:])
            out_sb = cpool.tile([P, npos * F], _FP32)

            for _r in range(repeat):
                for chunk_off, nchunks, poss in batches:
                    mt = mpool.tile([P, _DMA_BATCH_CHUNKS * F], _FP8)
                    nc.sync.dma_start(
                        out=mt[:, :nchunks * F],
                        in_=msg_d[:, chunk_off * F:(chunk_off + nchunks) * F])
                    for i, loc in poss:
                        s = schedule[i]
                        ps = ppool.tile([P, F], _FP32)
                        for w in range(s):
                            c = (loc + w) * F
                            nc.tensor.matmul(
                                out=ps[:],
                                lhsT=id_t[:],
                                rhs=mt[:, c:c + F],
                                start=(w == 0),
                                stop=(w == s - 1),
                            )
                        nc.scalar.copy(out=out_sb[:, i * F:(i + 1) * F],
                                       in_=ps[:])
                nc.gpsimd.dma_start(out=out_d[:], in_=out_sb[:])
    return nc


def _quantize_error_feedback(msg, order, starts, deg, np8=_NP8):
    """Per-node error-feedback fp8 quantization: q[e] = fp8(x[e] + carry),
    carry = running residual within the node's edge list, so the node sum of
    q tracks the exact sum to ~one final-carry. Returns q [E, F] fp8."""
    E = msg.shape[0]
    q = np.empty((E, F), dtype=np8)
    m = msg[order].astype(np.float32)
    n_nodes = len(deg)
    carry = np.zeros((n_nodes, F), np.float32)
    maxdeg = int(deg.max())
    base = starts[:-1]
    for w in range(maxdeg):
        sel = deg > w
        rows = base[sel] + w
        x = m[rows] + carry[sel]
        qx = x.astype(np8)
        q[order[rows]] = qx
        carry[sel] = x - qx.astype(np.float32)
    return q


def prepare_inputs_v3(msg: np.ndarray, edge_index: np.ndarray,
                      num_nodes: int = NUM_NODES, n_cores: int = N_CORES):
    """Returns (in_maps, schedule, node_tbl) where node_tbl[i, k, p] is the
    global node id owning output row (partition p, position i) on core k
    (-1 = dummy padding slot)."""
    E, feat = msg.shape
    assert feat == F

    dst = np.asarray(edge_index[1]).astype(np.int64)
    deg = np.bincount(dst, minlength=num_nodes)
    order = np.argsort(dst, kind="stable")         # edges grouped by dst
    starts = np.zeros(num_nodes + 1, dtype=np.int64)
    np.cumsum(deg, out=starts[1:])

    q = _quantize_error_feedback(np.asarray(msg), order, starts, deg)
    q_pad = np.concatenate([q, np.zeros((1, F), _NP8)], axis=0)

    # edge id per (node, slot): eid[n, w] = w-th edge of node n (E = none)
    maxdeg = int(deg.max())
    eid = np.full((num_nodes + 1, maxdeg), E, dtype=np.int64)
    pos_in_node = np.arange(E) - starts[dst[order]]
    eid[dst[order], pos_in_node] = order

    # degree-sorted node table: 176 dummy slots first, then nodes ascending
    node_order = np.argsort(deg, kind="stable")
    node_tbl = np.full(NODE_SLOTS, -1, dtype=np.int64)
    node_tbl[NODE_SLOTS - num_nodes:] = node_order
    node_tbl = node_tbl.reshape(NPOS, n_cores, P)

    deg_pad = np.concatenate([deg, [0]])
    schedule = deg_pad[node_tbl].max(axis=(1, 2)).astype(np.int64)
    assert schedule.min() >= 1
    tot = int(schedule.sum())
    offs = np.zeros(NPOS + 1, dtype=np.int64)
    np.cumsum(schedule, out=offs[1:])

    msg_dev = np.zeros((n_cores, P, tot * F), dtype=_NP8)
    md = msg_dev.reshape(n_cores, P, tot, F)
    for i in range(NPOS):
        s = int(schedule[i])
        g = eid[node_tbl[i], :s]                   # [n_cores, P, s]
        md[:, :, offs[i]:offs[i] + s] = q_pad[g]

    ident = np.zeros((P, P), dtype=_NP8)
    np.fill_diagonal(ident, 1.0)

    in_maps = [{"msg": msg_dev[k], "ident": ident} for k in range(n_cores)]
    return in_maps, schedule, node_tbl


def kernel_v3(msg: np.ndarray, edge_index: np.ndarray) -> np.ndarray:
    msg = np.asarray(msg)
    edge_index = np.asarray(edge_index)

    in_maps, schedule, node_tbl = prepare_inputs_v3(msg, edge_index)
    nc = build_program_v3(schedule, N_CORES)
    res = run_bass_kernel_spmd(nc, in_maps, list(range(N_CORES)))
    out = np.zeros((NUM_NODES, F), dtype=np.float32)
    for k in range(N_CORES):
        o = res.results[k]["out"].reshape(P, NPOS, F)      # [p, i, f]
        nodes = node_tbl[:, k, :]                          # [i, p]
        valid = nodes >= 0
        out[nodes[valid]] = o.transpose(1, 0, 2)[valid]
    return out


# ---------------------------------------------------------------------------
# V4: grouped slot-major layout + fp8e4 DoubleRow.
#
# v3 measured ~55 ns/chunk: each self-loading matmul re-streams the 128-col
# identity through the weight port, and at N=64 that load is the critical
# path. v4 groups 8 positions per matmul (slot-major interleave, N=512
# moving columns) so one weight load covers 8 chunks, and uses fp8e4
# DoubleRow (two slot chunks packed per partition row) to stream 2 chunks
# per position per matmul. At N=512 the moving stream (~240 ns) exceeds
# even an un-elided 256-col DoubleRow weight load (~213 ns), so the weight
# path is hidden either way. The degree-sorted position order keeps the
# per-group slot cap S_g = max(s[i]) tight (~4% padding).
# ---------------------------------------------------------------------------

_NP8E4 = ml_dtypes.float8_e4m3
_FP8E4 = mybir.dt.float8e4
_GROUP = 8


def _v4_groups(schedule):
    """[(pos0, G, S)] covering positions; S rounded even for DoubleRow."""
    groups = []
    for pos0 in range(0, len(schedule), _GROUP):
        G = min(_GROUP, len(schedule) - pos0)
        S = int(max(schedule[pos0:pos0 + G]))
        if G * F >= 256:          # DoubleRow groups need even slot count
            S += S % 2
        groups.append((pos0, G, S))
    return groups


def build_program_v4(groups, n_cores: int = N_CORES,
                     repeat: int = 1) -> bass.Bass:
    """groups: [(pos0, G, S)]; msg is [P, sum(G*S)*F] fp8e4 slot-major
    within each group: col block (g, w, j) = slot-w chunk of position
    pos0+j. id2 is the DoubleRow identity [P, 2*P]; out [P, npos*F] f32."""
    npos = max(pos0 + G for pos0, G, _ in groups)
    tot = sum(G * S for _, G, S in groups)
    nc = bass.Bass("TRN2", target_bir_lowering=False, debug=False,
                   num_devices=n_cores)
    msg_d = nc.dram_tensor("msg", [P, tot * F], _FP8E4,
                           kind="ExternalInput").ap()
    id_d = nc.dram_tensor("ident", [P, 2 * P], _FP8E4,
                          kind="ExternalInput").ap()
    out_d = nc.dram_tensor("out", [P, npos * F], _FP32,
                           kind="ExternalOutput").ap()

    max_cols = max(G * S for _, G, S in groups) * F

    with tile.TileContext(nc) as tc:
        with (
            tc.tile_pool(name="const", bufs=1) as cpool,
            tc.tile_pool(name="msg", bufs=3) as mpool,
            tc.tile_pool(name="psum", bufs=4, space="PSUM") as ppool,
        ):
            id2_t = cpool.tile([P, 2 * P], _FP8E4)
            nc.sync.dma_start(out=id2_t[:], in_=id_d[:])
            out_sb = cpool.tile([P, npos * F], _FP32)

            id2_3d = id2_t[:].rearrange("p (t m) -> p t m", t=2)
            for _r in range(repeat):
                off = 0
                for pos0, G, S in groups:
                    ncols = G * S * F
                    mt = mpool.tile([P, max_cols], _FP8E4)
                    nc.sync.dma_start(out=mt[:, :ncols],
                                      in_=msg_d[:, off * F:off * F + ncols])
                    off += G * S
                    if G * F >= 256:
                        ps = ppool.tile([P, G * F], _FP32, tag="psdr")
                        w2 = 2 * G * F
                        for ww in range(S // 2):
                            nc.tensor.matmul(
                                out=ps[:],
                                lhsT=id2_3d,
                                rhs=mt[:, ww * w2:(ww + 1) * w2]
                                    .rearrange("p (t n) -> p t n", t=2),
                                start=(ww == 0),
                                stop=(ww == S // 2 - 1),
                                perf_mode=mybir.MatmulPerfMode.DoubleRow,
                            )
                    else:                    # tail group: single-row
                        ps = ppool.tile([P, G * F], _FP32, tag="pssr")
                        for w in range(S):
                            nc.tensor.matmul(
                                out=ps[:],
                                lhsT=id2_t[:, P:2 * P],   # plain identity
                                rhs=mt[:, w * G * F:(w + 1) * G * F],
                                start=(w == 0),
                                stop=(w == S - 1),
                            )
                    nc.scalar.copy(
                        out=out_sb[:, pos0 * F:(pos0 + G) * F], in_=ps[:])
                nc.gpsimd.dma_start(out=out_d[:], in_=out_sb[:])
    return nc


def prepare_inputs_v4(msg: np.ndarray, edge_index: np.ndarray,
                      num_nodes: int = NUM_NODES, n_cores: int = N_CORES):
    """Returns (in_maps, groups, node_tbl)."""
    E, feat = msg.shape
    assert feat == F

    dst = np.asarray(edge_index[1]).astype(np.int64)
    deg = np.bincount(dst, minlength=num_nodes)
    order = np.argsort(dst, kind="stable")
    starts = np.zeros(num_nodes + 1, dtype=np.int64)
    np.cumsum(deg, out=starts[1:])

    q = _quantize_error_feedback(np.asarray(msg), order, starts, deg,
                                 np8=_NP8E4)
    q_pad = np.concatenate([q, np.zeros((1, F), _NP8E4)], axis=0)

    maxdeg = int(deg.max())
    eid = np.full((num_nodes + 1, maxdeg), E, dtype=np.int64)
    pos_in_node = np.arange(E) - starts[dst[order]]
    eid[dst[order], pos_in_node] = order

    node_order = np.argsort(deg, kind="stable")
    node_tbl = np.full(NODE_SLOTS, -1, dtype=np.int64)
    node_tbl[NODE_SLOTS - num_nodes:] = node_order
    node_tbl = node_tbl.reshape(NPOS, n_cores, P)

    deg_pad = np.concatenate([deg, [0]])
    schedule = deg_pad[node_tbl].max(axis=(1, 2)).astype(np.int64)
    groups = _v4_groups(schedule)
    tot = sum(G * S for _, G, S in groups)

    msg_dev = np.zeros((n_cores, P, tot * F), dtype=_NP8E4)
    md = msg_dev.reshape(n_cores, P, tot, F)
    off = 0
    for pos0, G, S in groups:
        nodes_g = node_tbl[pos0:pos0 + G]            # [G, cores, P]
        gi = eid[nodes_g, :S]                        # [G, cores, P, S]
        # -> [cores, P, S, G]
        gi = gi.transpose(1, 2, 3, 0)
        md[:, :, off:off + S * G] = q_pad[gi].reshape(
            n_cores, P, S * G, F)
        off += S * G

    id2 = np.zeros((P, 2 * P), dtype=_NP8E4)
    k = np.arange(P)
    id2[k, k] = 1.0            # t = 0 plane
    id2[k, P + k] = 1.0        # t = 1 plane (also the plain identity slice)

    in_maps = [{"msg": msg_dev[c], "ident": id2} for c in range(n_cores)]
    return in_maps, groups, node_tbl


def kernel_v4(msg: np.ndarray, edge_index: np.ndarray) -> np.ndarray:
    msg = np.asarray(msg)
    edge_index = np.asarray(edge_index)

    in_maps, groups, node_tbl = prepare_inputs_v4(msg, edge_index)
    nc = build_program_v4(groups, N_CORES)
    res = run_bass_kernel_spmd(nc, in_maps, list(range(N_CORES)))
    out = np.zeros((NUM_NODES, F), dtype=np.float32)
    for k in range(N_CORES):
        o = res.results[k]["out"].reshape(P, NPOS, F)      # [p, i, f]
        nodes = node_tbl[:, k, :]                          # [i, p]
        valid = nodes >= 0
        out[nodes[valid]] = o.transpose(1, 0, 2)[valid]
    return out


# ---------------------------------------------------------------------------
# V5: ragged slot compaction + bf16 output.
#
# v4 pads every position of a group to the group-max slot count (+11% DMA).
# v5 orders each group's positions by descending slot count, so slot slab
# w only carries the prefix of positions still active (width W_w). Each
# DoubleRow matmul pair accumulates into a shrinking PREFIX of the psum
# tile (per-element has_written semantics make partial accumulation safe;
# pair 0 has full width and carries start=True). Output is stored as bf16
# (host upcasts), halving the write traffic; the added rounding (~2e-3 of
# scale) is well inside the error budget.
# ---------------------------------------------------------------------------

_BF16 = mybir.dt.bfloat16
_NPBF16 = ml_dtypes.bfloat16


def build_program_v5(groups, n_cores: int = N_CORES,
                     repeat: int = 1) -> bass.Bass:
    """groups: [(pos0, G, widths)]; for a DoubleRow group (G*F >= 256),
    widths[ww] = position-prefix width of slot pair (2ww, 2ww+1) and the
    msg slab for the pair is [2 planes x widths[ww]*F] columns; for a
    single-row group widths[w] = width of slot w (one plane)."""
    npos = max(pos0 + G for pos0, G, _ in groups)

    def group_chunks(G, widths):
        return sum((2 if G * F >= 256 else 1) * W for W in widths)

    tot = sum(group_chunks(G, ws) for _, G, ws in groups)
    nc = bass.Bass("TRN2", target_bir_lowering=False, debug=False,
                   num_devices=n_cores)
    msg_d = nc.dram_tensor("msg", [P, tot * F], _FP8E4,
                           kind="ExternalInput").ap()
    id_d = nc.dram_tensor("ident", [P, 2 * P], _FP8E4,
                          kind="ExternalInput").ap()
    out_d = nc.dram_tensor("out", [P, npos * F], _BF16,
                           kind="ExternalOutput").ap()

    max_cols = max(group_chunks(G, ws) for _, G, ws in groups) * F

    with tile.TileContext(nc) as tc:
        with (
            tc.tile_pool(name="const", bufs=1) as cpool,
            tc.tile_pool(name="msg", bufs=3) as mpool,
            tc.tile_pool(name="outp", bufs=2) as opool,
            tc.tile_pool(name="psum", bufs=4, space="PSUM") as ppool,
        ):
            id2_t = cpool.tile([P, 2 * P], _FP8E4)
            nc.sync.dma_start(out=id2_t[:], in_=id_d[:])

            id2_3d = id2_t[:].rearrange("p (t m) -> p t m", t=2)
            if repeat == 1:
                # single-shot: ~3.4us of junk matmuls while the first msg
                # load streams, so the PE HAM clock-gate is released (1.2 ->
                # 2.4 GHz) before the real accumulation chains start
                wps = ppool.tile([P, _GROUP * F], _FP32, tag=f"ps{_GROUP}")
                for _ in range(32):
                    nc.tensor.matmul(out=wps[:, :P], lhsT=id2_t[:, P:2 * P],
                                     rhs=id2_t[:, :P], start=True, stop=True)
            dma_rings = [nc.sync, nc.scalar]
            for _r in range(repeat):
                out_sb = opool.tile([P, npos * F], _BF16)
                off = 0
                for gi_, (pos0, G, widths) in enumerate(groups):
                    ncols = group_chunks(G, widths) * F
                    mt = mpool.tile([P, max_cols], _FP8E4)
                    dma_rings[gi_ % 2].dma_start(
                        out=mt[:, :ncols],
                        in_=msg_d[:, off * F:off * F + ncols])
                    off += group_chunks(G, widths)
                    ps = ppool.tile([P, G * F], _FP32, tag=f"ps{G}")
                    c = 0
                    if G * F >= 256:
                        for ww, W in enumerate(widths):
                            nc.tensor.matmul(
                                out=ps[:, :W * F],
                                lhsT=id2_3d,
                                rhs=mt[:, c:c + 2 * W * F]
                                    .rearrange("p (t n) -> p t n", t=2),
                                start=(ww == 0),
                                stop=(ww == len(widths) - 1),
                                perf_mode=mybir.MatmulPerfMode.DoubleRow,
                                skip_group_check=True,
                            )
                            c += 2 * W * F
                    else:
                        for w, W in enumerate(widths):
                            nc.tensor.matmul(
                                out=ps[:, :W * F],
                                lhsT=id2_t[:, P:P + P],
                                rhs=mt[:, c:c + W * F],
                                start=(w == 0),
                                stop=(w == len(widths) - 1),
                                skip_group_check=True,
                            )
                            c += W * F
                    nc.scalar.copy(
                        out=out_sb[:, pos0 * F:(pos0 + G) * F], in_=ps[:])
                nc.gpsimd.dma_start(out=out_d[:], in_=out_sb[:])
    return nc


def prepare_inputs_v5(msg: np.ndarray, edge_index: np.ndarray,
                      num_nodes: int = NUM_NODES, n_cores: int = N_CORES):
    """Returns (in_maps, groups, node_tbl); groups as build_program_v5."""
    E, feat = msg.shape
    assert feat == F

    dst = np.asarray(edge_index[1]).astype(np.int64)
    deg = np.bincount(dst, minlength=num_nodes)
    order = np.argsort(dst, kind="stable")
    starts = np.zeros(num_nodes + 1, dtype=np.int64)
    np.cumsum(deg, out=starts[1:])

    q = _quantize_error_feedback(np.asarray(msg), order, starts, deg,
                                 np8=_NP8E4)
    q_pad = np.concatenate([q, np.zeros((1, F), _NP8E4)], axis=0)

    maxdeg = int(deg.max())
    # one spare all-empty slot column so odd pair planes can index slot S
    eid = np.full((num_nodes + 1, maxdeg + 1), E, dtype=np.int64)
    pos_in_node = np.arange(E) - starts[dst[order]]
    eid[dst[order], pos_in_node] = order

    node_order = np.argsort(deg, kind="stable")
    node_tbl = np.full(NODE_SLOTS, -1, dtype=np.int64)
    node_tbl[NODE_SLOTS - num_nodes:] = node_order
    node_tbl = node_tbl.reshape(NPOS, n_cores, P)

    deg_pad = np.concatenate([deg, [0]])
    schedule = deg_pad[node_tbl].max(axis=(1, 2)).astype(np.int64)

    # within each group of _GROUP positions, order positions by descending
    # slot count so slot slabs are prefix-shaped
    groups = []
    for pos0 in range(0, NPOS, _GROUP):
        G = min(_GROUP, NPOS - pos0)
        perm = np.argsort(-schedule[pos0:pos0 + G], kind="stable")
        node_tbl[pos0:pos0 + G] = node_tbl[pos0 + perm]
        schedule[pos0:pos0 + G] = schedule[pos0 + perm]
        s = schedule[pos0:pos0 + G]
        S = int(s[0])
        if G * F >= 256:
            widths = [int((s >= 2 * ww + 1).sum())
                      for ww in range((S + 1) // 2)]
        else:
            widths = [int((s >= w + 1).sum()) for w in range(S)]
        groups.append((pos0, G, widths))

    def group_chunks(G, widths):
        return sum((2 if G * F >= 256 else 1) * W for W in widths)

    tot = sum(group_chunks(G, ws) for _, G, ws in groups)

    msg_dev = np.zeros((n_cores, P, tot * F), dtype=_NP8E4)
    md = msg_dev.reshape(n_cores, P, tot, F)
    off = 0
    for pos0, G, widths in groups:
        nplanes = 2 if G * F >= 256 else 1
        for ww, W in enumerate(widths):
            nodes_w = node_tbl[pos0:pos0 + W]            # [W, cores, P]
            slots = [nplanes * ww + t for t in range(nplanes)]
            gi = eid[nodes_w][:, :, :, slots]            # [W, cores, P, t]
            gi = gi.transpose(1, 2, 3, 0)                # [cores, P, t, W]
            md[:, :, off:off + nplanes * W] = q_pad[gi].reshape(
                n_cores, P, nplanes * W, F)
            off += nplanes * W

    id2 = np.zeros((P, 2 * P), dtype=_NP8E4)
    k = np.arange(P)
    id2[k, k] = 1.0
    id2[k, P + k] = 1.0

    in_maps = [{"msg": msg_dev[c], "ident": id2} for c in range(n_cores)]
    return in_maps, groups, node_tbl


def kernel_v5(msg: np.ndarray, edge_index: np.ndarray) -> np.ndarray:
    msg = np.asarray(msg)
    edge_index = np.asarray(edge_index)

    in_maps, groups, node_tbl = prepare_inputs_v5(msg, edge_index)
    nc = build_program_v5(groups, N_CORES)
    res = run_bass_kernel_spmd(nc, in_maps, list(range(N_CORES)))
    out = np.zeros((NUM_NODES, F), dtype=np.float32)
    for k in range(N_CORES):
        o = res.results[k]["out"].astype(np.float32).reshape(P, NPOS, F)
        nodes = node_tbl[:, k, :]                          # [i, p]
        valid = nodes >= 0
        out[nodes[valid]] = o.transpose(1, 0, 2)[valid]
    return out


def kernel(msg: np.ndarray, edge_index: np.ndarray) -> np.ndarray:
    return kernel_v5(msg, edge_index)


# revision 11
# speedup vs baseline: 1.1929x; 1.1929x over previous
"""Trainium2 Bass kernel for segment_sum (GAT reduce-sum stage).

out[n, :] = sum over edges e with dst[e] == n of msg[e, :],  n in [0, 50000).

Strategy v3 (8 NeuronCores, SPMD single program): identity-stationary slot
accumulation.

  - Host sorts nodes by degree and packs 128 consecutive (sorted) nodes per
    tile; a "position" is 8 consecutive tiles, one per core, so all cores run
    an identical schedule s[i] = max degree within position i. Each node's
    edges fill slots w = 0..deg-1 of its partition row (zero-padded to s[i]).
    Because degrees are dense small integers, sorting makes the padding ~1%
    (vs ~24% for the per-node class scheme this replaces).
  - msg ships as fp8 e3m4 quantized with per-node error feedback (the
    quantization residual of slot w is carried into slot w+1), so the
    on-device fp32 sum of the quantized slots tracks the exact per-node sum;
    measured rel err ~4e-3 vs the 2e-2 budget. DMA traffic is ~10 MB/core.
  - Device: the PE stationary operand is a [128, 128] identity, loaded from
    the first matmul and identical for every matmul in the program. Each
    matmul accumulates one [128 nodes, 64 feat] slot chunk into PSUM:
    out[m, n] += sum_p I[p, m] * chunk[p, n] = chunk[m, n]. The moving side
    streams 64 columns (~27 ns/chunk); there is no per-tile weight churn,
    no on-device one-hot build, and DVE stays idle.
  - Output tiles accumulate into a persistent SBUF strip; one large store
    per pass. No collectives: cores own disjoint node sets; host scatters.
"""

import numpy as np
import ml_dtypes

import concourse.tile as tile
from concourse import bass, mybir
from concourse.bass_utils import run_bass_kernel_spmd
from concourse.vector_clock import ScopedClock

P = 128          # partitions / tile node count
F = 64           # feature dim
N_CORES = 8
NUM_NODES = 50000
NPOS = -(-NUM_NODES // (P * N_CORES))          # 49 positions per core
NODE_SLOTS = NPOS * P * N_CORES                # 50176 (176 dummy slots)

_FP8 = mybir.dt.float8e3                       # e3m4
_NP8 = ml_dtypes.float8_e3m4
_FP32 = mybir.dt.float32

_DMA_BATCH_CHUNKS = 256    # 256 * 64 B = 16 KiB / partition per load (~2 MB)

_MAX_INST_WAITS = 1


def _split_excess_waits(nc, max_waits: int = _MAX_INST_WAITS):
    """This walrus build rejects instructions carrying more than `max_waits`
    sem waits ("Too many sync wait commands"), but Tile's wait pass piles
    every needed proc wait onto the consuming instruction. Hoist the excess
    onto wait-only EventSemaphore instructions inserted just before, on the
    same engine queue (same semantics: queue is in-order)."""
    n = 0
    for f in nc.m.functions:
        for b in f.blocks:
            il = b.instructions
            out = []
            changed = False
            for inst in il:
                si = inst.sync_info
                if si is not None and si.on_wait and len(si.on_wait) > max_waits:
                    waits = list(si.on_wait)
                    extra, keep = waits[:-max_waits], waits[-max_waits:]
                    for i in range(0, len(extra), max_waits):
                        ev = mybir.InstEventSemaphore(
                            name=f"{inst.name}-wsplit{n}",
                            engine=inst.engine,
                            ins=[],
                            outs=[],
                            sync_info=mybir.SyncInfo(
                                on_wait=extra[i:i + max_waits], on_update=[]),
                        )
                        n += 1
                        out.append(ev)
                    inst.sync_info = mybir.SyncInfo(
                        on_wait=keep, on_update=list(si.on_update))
                    changed = True
                out.append(inst)
            if changed:
                b.instructions = out


def _patched_drain_and_barrier(self, tick_clock, wait_clock):
    nc = self.nc
    probe = nc.sync.nop(nofuse=True, hint="drain_waits")
    wait_clock.add_sem_waits(probe.ins, ScopedClock({None: tick_clock.global_clock}))
    si = probe.ins.sync_info
    waits = list(si.on_wait) if si is not None else []
    if si is not None:
        del si.on_wait[:]
    by_name = {h.name: h for h in self.sems.allocated().values()}
    for w in waits:
        assert w.wait_reg is None
        nc.sync.wait_ge(by_name[w.ant_name], w.wait_value)
    nc.sync.drain()

    nc.all_engine_barrier()
    popped = nc._tile_sem_poison_stack.pop()
    assert popped is self._sem_poison
    nc.clear_and_free_semaphores(list(self.sems.allocated().values()))
    nc.all_engine_barrier()

    _split_excess_waits(nc)


tile.TileContext._drain_and_barrier = _patched_drain_and_barrier


def build_program_v3(schedule, n_cores: int = N_CORES,
                     repeat: int = 1) -> bass.Bass:
    """schedule: per-position slot counts s[i] (same for every core).

    Inputs per core: msg [P, TOT*F] fp8 (TOT = sum(s); position i occupies
    chunk columns off[i]..off[i]+s[i], chunk w = slot-w edge of the
    partition's node), ident [P, P] fp8. Output: [P, NPOS*F] fp32
    (position-major strip; host transposes/scatters).

    repeat > 1 re-runs the whole body (for steady-state timing via the
    T(N) slope; each repeat writes the same output)."""
    npos = len(schedule)
    tot = int(sum(schedule))
    nc = bass.Bass("TRN2", target_bir_lowering=False, debug=False,
                   num_devices=n_cores)
    msg_d = nc.dram_tensor("msg", [P, tot * F], _FP8,
                           kind="ExternalInput").ap()
    id_d = nc.dram_tensor("ident", [P, P], _FP8, kind="ExternalInput").ap()
    out_d = nc.dram_tensor("out", [P, npos * F], _FP32,
                           kind="ExternalOutput").ap()

    # batch positions into DMA loads of <= _DMA_BATCH_CHUNKS chunks
    batches = []          # (chunk_off, nchunks, [(pos, local_chunk_off), ...])
    cur_off, cur_n, cur_pos = 0, 0, []
    off = 0
    for i, s in enumerate(schedule):
        if cur_n + s > _DMA_BATCH_CHUNKS and cur_n > 0:
            batches.append((cur_off, cur_n, cur_pos))
            cur_off, cur_n, cur_pos = off, 0, []
        cur_pos.append((i, cur_n))
        cur_n += s
        off += s
    if cur_n:
        batches.append((cur_off, cur_n, cur_pos))

    with tile.TileContext(nc) as tc:
        with (
            tc.tile_pool(name="const", bufs=1) as cpool,
            tc.tile_pool(name="msg", bufs=3) as mpool,
            tc.tile_pool(name="psum", bufs=8, space="PSUM") as ppool,
        ):
            id_t = cpool.tile([P, P], _FP8)
            nc.sync.dma_start(out=id_t[:], in_=id_d[:])
            out_sb = cpool.tile([P, npos * F], _FP32)

            for _r in range(repeat):
                for chunk_off, nchunks, poss in batches:
                    mt = mpool.tile([P, _DMA_BATCH_CHUNKS * F], _FP8)
                    nc.sync.dma_start(
                        out=mt[:, :nchunks * F],
                        in_=msg_d[:, chunk_off * F:(chunk_off + nchunks) * F])
                    for i, loc in poss:
                        s = schedule[i]
                        ps = ppool.tile([P, F], _FP32)
                        for w in range(s):
                            c = (loc + w) * F
                            nc.tensor.matmul(
                                out=ps[:],
                                lhsT=id_t[:],
                                rhs=mt[:, c:c + F],
                                start=(w == 0),
                                stop=(w == s - 1),
                            )
                        nc.scalar.copy(out=out_sb[:, i * F:(i + 1) * F],
                                       in_=ps[:])
                nc.gpsimd.dma_start(out=out_d[:], in_=out_sb[:])
    return nc


def _quantize_error_feedback(msg, order, starts, deg, np8=_NP8):
    """Per-node error-feedback fp8 quantization: q[e] = fp8(x[e] + carry),
    carry = running residual within the node's edge list, so the node sum of
    q tracks the exact sum to ~one final-carry. Returns q [E, F] fp8."""
    E = msg.shape[0]
    q = np.empty((E, F), dtype=np8)
    m = msg[order].astype(np.float32)
    n_nodes = len(deg)
    carry = np.zeros((n_nodes, F), np.float32)
    maxdeg = int(deg.max())
    base = starts[:-1]
    for w in range(maxdeg):
        sel = deg > w
        rows = base[sel] + w
        x = m[rows] + carry[sel]
        qx = x.astype(np8)
        q[order[rows]] = qx
        carry[sel] = x - qx.astype(np.float32)
    return q


def prepare_inputs_v3(msg: np.ndarray, edge_index: np.ndarray,
                      num_nodes: int = NUM_NODES, n_cores: int = N_CORES):
    """Returns (in_maps, schedule, node_tbl) where node_tbl[i, k, p] is the
    global node id owning output row (partition p, position i) on core k
    (-1 = dummy padding slot)."""
    E, feat = msg.shape
    assert feat == F

    dst = np.asarray(edge_index[1]).astype(np.int64)
    deg = np.bincount(dst, minlength=num_nodes)
    order = np.argsort(dst, kind="stable")         # edges grouped by dst
    starts = np.zeros(num_nodes + 1, dtype=np.int64)
    np.cumsum(deg, out=starts[1:])

    q = _quantize_error_feedback(np.asarray(msg), order, starts, deg)
    q_pad = np.concatenate([q, np.zeros((1, F), _NP8)], axis=0)

    # edge id per (node, slot): eid[n, w] = w-th edge of node n (E = none)
    maxdeg = int(deg.max())
    eid = np.full((num_nodes + 1, maxdeg), E, dtype=np.int64)
    pos_in_node = np.arange(E) - starts[dst[order]]
    eid[dst[order], pos_in_node] = order

    # degree-sorted node table: 176 dummy slots first, then nodes ascending
    node_order = np.argsort(deg, kind="stable")
    node_tbl = np.full(NODE_SLOTS, -1, dtype=np.int64)
    node_tbl[NODE_SLOTS - num_nodes:] = node_order
    node_tbl = node_tbl.reshape(NPOS, n_cores, P)

    deg_pad = np.concatenate([deg, [0]])
    schedule = deg_pad[node_tbl].max(axis=(1, 2)).astype(np.int64)
    assert schedule.min() >= 1
    tot = int(schedule.sum())
    offs = np.zeros(NPOS + 1, dtype=np.int64)
    np.cumsum(schedule, out=offs[1:])

    msg_dev = np.zeros((n_cores, P, tot * F), dtype=_NP8)
    md = msg_dev.reshape(n_cores, P, tot, F)
    for i in range(NPOS):
        s = int(schedule[i])
        g = eid[node_tbl[i], :s]                   # [n_cores, P, s]
        md[:, :, offs[i]:offs[i] + s] = q_pad[g]

    ident = np.zeros((P, P), dtype=_NP8)
    np.fill_diagonal(ident, 1.0)

    in_maps = [{"msg": msg_dev[k], "ident": ident} for k in range(n_cores)]
    return in_maps, schedule, node_tbl


def kernel_v3(msg: np.ndarray, edge_index: np.ndarray) -> np.ndarray:
    msg = np.asarray(msg)
    edge_index = np.asarray(edge_index)

    in_maps, schedule, node_tbl = prepare_inputs_v3(msg, edge_index)
    nc = build_program_v3(schedule, N_CORES)
    res = run_bass_kernel_spmd(nc, in_maps, list(range(N_CORES)))
    out = np.zeros((NUM_NODES, F), dtype=np.float32)
    for k in range(N_CORES):
        o = res.results[k]["out"].reshape(P, NPOS, F)      # [p, i, f]
        nodes = node_tbl[:, k, :]                          # [i, p]
        valid = nodes >= 0
        out[nodes[valid]] = o.transpose(1, 0, 2)[valid]
    return out


# ---------------------------------------------------------------------------
# V4: grouped slot-major layout + fp8e4 DoubleRow.
#
# v3 measured ~55 ns/chunk: each self-loading matmul re-streams the 128-col
# identity through the weight port, and at N=64 that load is the critical
# path. v4 groups 8 positions per matmul (slot-major interleave, N=512
# moving columns) so one weight load covers 8 chunks, and uses fp8e4
# DoubleRow (two slot chunks packed per partition row) to stream 2 chunks
# per position per matmul. At N=512 the moving stream (~240 ns) exceeds
# even an un-elided 256-col DoubleRow weight load (~213 ns), so the weight
# path is hidden either way. The degree-sorted position order keeps the
# per-group slot cap S_g = max(s[i]) tight (~4% padding).
# ---------------------------------------------------------------------------

_NP8E4 = ml_dtypes.float8_e4m3
_FP8E4 = mybir.dt.float8e4
_GROUP = 8


def _v4_groups(schedule):
    """[(pos0, G, S)] covering positions; S rounded even for DoubleRow."""
    groups = []
    for pos0 in range(0, len(schedule), _GROUP):
        G = min(_GROUP, len(schedule) - pos0)
        S = int(max(schedule[pos0:pos0 + G]))
        if G * F >= 256:          # DoubleRow groups need even slot count
            S += S % 2
        groups.append((pos0, G, S))
    return groups


def build_program_v4(groups, n_cores: int = N_CORES,
                     repeat: int = 1) -> bass.Bass:
    """groups: [(pos0, G, S)]; msg is [P, sum(G*S)*F] fp8e4 slot-major
    within each group: col block (g, w, j) = slot-w chunk of position
    pos0+j. id2 is the DoubleRow identity [P, 2*P]; out [P, npos*F] f32."""
    npos = max(pos0 + G for pos0, G, _ in groups)
    tot = sum(G * S for _, G, S in groups)
    nc = bass.Bass("TRN2", target_bir_lowering=False, debug=False,
                   num_devices=n_cores)
    msg_d = nc.dram_tensor("msg", [P, tot * F], _FP8E4,
                           kind="ExternalInput").ap()
    id_d = nc.dram_tensor("ident", [P, 2 * P], _FP8E4,
                          kind="ExternalInput").ap()
    out_d = nc.dram_tensor("out", [P, npos * F], _FP32,
                           kind="ExternalOutput").ap()

    max_cols = max(G * S for _, G, S in groups) * F

    with tile.TileContext(nc) as tc:
        with (
            tc.tile_pool(name="const", bufs=1) as cpool,
            tc.tile_pool(name="msg", bufs=3) as mpool,
            tc.tile_pool(name="psum", bufs=4, space="PSUM") as ppool,
        ):
            id2_t = cpool.tile([P, 2 * P], _FP8E4)
            nc.sync.dma_start(out=id2_t[:], in_=id_d[:])
            out_sb = cpool.tile([P, npos * F], _FP32)

            id2_3d = id2_t[:].rearrange("p (t m) -> p t m", t=2)
            for _r in range(repeat):
                off = 0
                for pos0, G, S in groups:
                    ncols = G * S * F
                    mt = mpool.tile([P, max_cols], _FP8E4)
                    nc.sync.dma_start(out=mt[:, :ncols],
                                      in_=msg_d[:, off * F:off * F + ncols])
                    off += G * S
                    if G * F >= 256:
                        ps = ppool.tile([P, G * F], _FP32, tag="psdr")
                        w2 = 2 * G * F
                        for ww in range(S // 2):
                            nc.tensor.matmul(
                                out=ps[:],
                                lhsT=id2_3d,
                                rhs=mt[:, ww * w2:(ww + 1) * w2]
                                    .rearrange("p (t n) -> p t n", t=2),
                                start=(ww == 0),
                                stop=(ww == S // 2 - 1),
                                perf_mode=mybir.MatmulPerfMode.DoubleRow,
                            )
                    else:                    # tail group: single-row
                        ps = ppool.tile([P, G * F], _FP32, tag="pssr")
                        for w in range(S):
                            nc.tensor.matmul(
                                out=ps[:],
                                lhsT=id2_t[:, P:2 * P],   # plain identity
                                rhs=mt[:, w * G * F:(w + 1) * G * F],
                                start=(w == 0),
                                stop=(w == S - 1),
                            )
                    nc.scalar.copy(
                        out=out_sb[:, pos0 * F:(pos0 + G) * F], in_=ps[:])
                nc.gpsimd.dma_start(out=out_d[:], in_=out_sb[:])
    return nc


def prepare_inputs_v4(msg: np.ndarray, edge_index: np.ndarray,
                      num_nodes: int = NUM_NODES, n_cores: int = N_CORES):
    """Returns (in_maps, groups, node_tbl)."""
    E, feat = msg.shape
    assert feat == F

    dst = np.asarray(edge_index[1]).astype(np.int64)
    deg = np.bincount(dst, minlength=num_nodes)
    order = np.argsort(dst, kind="stable")
    starts = np.zeros(num_nodes + 1, dtype=np.int64)
    np.cumsum(deg, out=starts[1:])

    q = _quantize_error_feedback(np.asarray(msg), order, starts, deg,
                                 np8=_NP8E4)
    q_pad = np.concatenate([q, np.zeros((1, F), _NP8E4)], axis=0)

    maxdeg = int(deg.max())
    eid = np.full((num_nodes + 1, maxdeg), E, dtype=np.int64)
    pos_in_node = np.arange(E) - starts[dst[order]]
    eid[dst[order], pos_in_node] = order

    node_order = np.argsort(deg, kind="stable")
    node_tbl = np.full(NODE_SLOTS, -1, dtype=np.int64)
    node_tbl[NODE_SLOTS - num_nodes:] = node_order
    node_tbl = node_tbl.reshape(NPOS, n_cores, P)

    deg_pad = np.concatenate([deg, [0]])
    schedule = deg_pad[node_tbl].max(axis=(1, 2)).astype(np.int64)
    groups = _v4_groups(schedule)
    tot = sum(G * S for _, G, S in groups)

    msg_dev = np.zeros((n_cores, P, tot * F), dtype=_NP8E4)
    md = msg_dev.reshape(n_cores, P, tot, F)
    off = 0
    for pos0, G, S in groups:
        nodes_g = node_tbl[pos0:pos0 + G]            # [G, cores, P]
        gi = eid[nodes_g, :S]                        # [G, cores, P, S]
        # -> [cores, P, S, G]
        gi = gi.transpose(1, 2, 3, 0)
        md[:, :, off:off + S * G] = q_pad[gi].reshape(
            n_cores, P, S * G, F)
        off += S * G

    id2 = np.zeros((P, 2 * P), dtype=_NP8E4)
    k = np.arange(P)
    id2[k, k] = 1.0            # t = 0 plane
    id2[k, P + k] = 1.0        # t = 1 plane (also the plain identity slice)

    in_maps = [{"msg": msg_dev[c], "ident": id2} for c in range(n_cores)]
    return in_maps, groups, node_tbl


def kernel_v4(msg: np.ndarray, edge_index: np.ndarray) -> np.ndarray:
    msg = np.asarray(msg)
    edge_index = np.asarray(edge_index)

    in_maps, groups, node_tbl = prepare_inputs_v4(msg, edge_index)
    nc = build_program_v4(groups, N_CORES)
    res = run_bass_kernel_spmd(nc, in_maps, list(range(N_CORES)))
    out = np.zeros((NUM_NODES, F), dtype=np.float32)
    for k in range(N_CORES):
        o = res.results[k]["out"].reshape(P, NPOS, F)      # [p, i, f]
        nodes = node_tbl[:, k, :]                          # [i, p]
        valid = nodes >= 0
        out[nodes[valid]] = o.transpose(1, 0, 2)[valid]
    return out


# ---------------------------------------------------------------------------
# V5: ragged slot compaction + bf16 output.
#
# v4 pads every position of a group to the group-max slot count (+11% DMA).
# v5 orders each group's positions by descending slot count, so slot slab
# w only carries the prefix of positions still active (width W_w). Each
# DoubleRow matmul pair accumulates into a shrinking PREFIX of the psum
# tile (per-element has_written semantics make partial accumulation safe;
# pair 0 has full width and carries start=True). Output is stored as bf16
# (host upcasts), halving the write traffic; the added rounding (~2e-3 of
# scale) is well inside the error budget.
# ---------------------------------------------------------------------------

_BF16 = mybir.dt.bfloat16
_NPBF16 = ml_dtypes.bfloat16


def build_program_v5(groups, n_cores: int = N_CORES,
                     repeat: int = 1) -> bass.Bass:
    """groups: [(pos0, G, widths)]; for a DoubleRow group (G*F >= 256),
    widths[ww] = position-prefix width of slot pair (2ww, 2ww+1) and the
    msg slab for the pair is [2 planes x widths[ww]*F] columns; for a
    single-row group widths[w] = width of slot w (one plane)."""
    npos = max(pos0 + G for pos0, G, _ in groups)

    def group_chunks(G, widths):
        return sum((2 if G * F >= 256 else 1) * W for W in widths)

    tot = sum(group_chunks(G, ws) for _, G, ws in groups)
    nc = bass.Bass("TRN2", target_bir_lowering=False, debug=False,
                   num_devices=n_cores)
    msg_d = nc.dram_tensor("msg", [P, tot * F], _FP8E4,
                           kind="ExternalInput").ap()
    id_d = nc.dram_tensor("ident", [P, 2 * P], _FP8E4,
                          kind="ExternalInput").ap()
    out_d = nc.dram_tensor("out", [P, npos * F], _BF16,
                           kind="ExternalOutput").ap()

    max_cols = max(group_chunks(G, ws) for _, G, ws in groups) * F

    with tile.TileContext(nc) as tc:
        with (
            tc.tile_pool(name="const", bufs=1) as cpool,
            tc.tile_pool(name="msg", bufs=4) as mpool,
            tc.tile_pool(name="outp", bufs=2) as opool,
            tc.tile_pool(name="psum", bufs=4, space="PSUM") as ppool,
        ):
            id2_t = cpool.tile([P, 2 * P], _FP8E4)
            nc.sync.dma_start(out=id2_t[:], in_=id_d[:])

            id2_3d = id2_t[:].rearrange("p (t m) -> p t m", t=2)
            if repeat == 1:
                # single-shot: ~3.4us of junk matmuls while the first msg
                # load streams, so the PE HAM clock-gate is released (1.2 ->
                # 2.4 GHz) before the real accumulation chains start
                wps = ppool.tile([P, _GROUP * F], _FP32, tag=f"ps{_GROUP}")
                for _ in range(32):
                    nc.tensor.matmul(out=wps[:, :P], lhsT=id2_t[:, P:2 * P],
                                     rhs=id2_t[:, :P], start=True, stop=True)
            dma_rings = [nc.sync, nc.gpsimd]
            for _r in range(repeat):
                out_sb = opool.tile([P, npos * F], _BF16)
                off = 0
                for gi_, (pos0, G, widths) in enumerate(groups):
                    ncols = group_chunks(G, widths) * F
                    mt = mpool.tile([P, max_cols], _FP8E4)
                    dma_rings[gi_ % 2].dma_start(
                        out=mt[:, :ncols],
                        in_=msg_d[:, off * F:off * F + ncols])
                    off += group_chunks(G, widths)
                    ps = ppool.tile([P, G * F], _FP32, tag=f"ps{G}")
                    c = 0
                    if G * F >= 256:
                        for ww, W in enumerate(widths):
                            nc.tensor.matmul(
                                out=ps[:, :W * F],
                                lhsT=id2_3d,
                                rhs=mt[:, c:c + 2 * W * F]
                                    .rearrange("p (t n) -> p t n", t=2),
                                start=(ww == 0),
                                stop=(ww == len(widths) - 1),
                                perf_mode=mybir.MatmulPerfMode.DoubleRow,
                                skip_group_check=True,
                            )
                            c += 2 * W * F
                    else:
                        for w, W in enumerate(widths):
                            nc.tensor.matmul(
                                out=ps[:, :W * F],
                                lhsT=id2_t[:, P:P + P],
                                rhs=mt[:, c:c + W * F],
                                start=(w == 0),
                                stop=(w == len(widths) - 1),
                                skip_group_check=True,
                            )
                            c += W * F
                    nc.scalar.copy(
                        out=out_sb[:, pos0 * F:(pos0 + G) * F], in_=ps[:])
                nc.gpsimd.dma_start(out=out_d[:], in_=out_sb[:])
    return nc


def prepare_inputs_v5(msg: np.ndarray, edge_index: np.ndarray,
                      num_nodes: int = NUM_NODES, n_cores: int = N_CORES):
    """Returns (in_maps, groups, node_tbl); groups as build_program_v5."""
    E, feat = msg.shape
    assert feat == F

    dst = np.asarray(edge_index[1]).astype(np.int64)
    deg = np.bincount(dst, minlength=num_nodes)
    order = np.argsort(dst, kind="stable")
    starts = np.zeros(num_nodes + 1, dtype=np.int64)
    np.cumsum(deg, out=starts[1:])

    q = _quantize_error_feedback(np.asarray(msg), order, starts, deg,
                                 np8=_NP8E4)
    q_pad = np.concatenate([q, np.zeros((1, F), _NP8E4)], axis=0)

    maxdeg = int(deg.max())
    # one spare all-empty slot column so odd pair planes can index slot S
    eid = np.full((num_nodes + 1, maxdeg + 1), E, dtype=np.int64)
    pos_in_node = np.arange(E) - starts[dst[order]]
    eid[dst[order], pos_in_node] = order

    node_order = np.argsort(deg, kind="stable")
    node_tbl = np.full(NODE_SLOTS, -1, dtype=np.int64)
    node_tbl[NODE_SLOTS - num_nodes:] = node_order
    node_tbl = node_tbl.reshape(NPOS, n_cores, P)

    deg_pad = np.concatenate([deg, [0]])
    schedule = deg_pad[node_tbl].max(axis=(1, 2)).astype(np.int64)

    # within each group of _GROUP positions, order positions by descending
    # slot count so slot slabs are prefix-shaped
    groups = []
    for pos0 in range(0, NPOS, _GROUP):
        G = min(_GROUP, NPOS - pos0)
        perm = np.argsort(-schedule[pos0:pos0 + G], kind="stable")
        node_tbl[pos0:pos0 + G] = node_tbl[pos0 + perm]
        schedule[pos0:pos0 + G] = schedule[pos0 + perm]
        s = schedule[pos0:pos0 + G]
        S = int(s[0])
        if G * F >= 256:
            widths = [int((s >= 2 * ww + 1).sum())
                      for ww in range((S + 1) // 2)]
        else:
            widths = [int((s >= w + 1).sum()) for w in range(S)]
        groups.append((pos0, G, widths))

    def group_chunks(G, widths):
        return sum((2 if G * F >= 256 else 1) * W for W in widths)

    tot = sum(group_chunks(G, ws) for _, G, ws in groups)

    msg_dev = np.zeros((n_cores, P, tot * F), dtype=_NP8E4)
    md = msg_dev.reshape(n_cores, P, tot, F)
    off = 0
    for pos0, G, widths in groups:
        nplanes = 2 if G * F >= 256 else 1
        for ww, W in enumerate(widths):
            nodes_w = node_tbl[pos0:pos0 + W]            # [W, cores, P]
            slots = [nplanes * ww + t for t in range(nplanes)]
            gi = eid[nodes_w][:, :, :, slots]            # [W, cores, P, t]
            gi = gi.transpose(1, 2, 3, 0)                # [cores, P, t, W]
            md[:, :, off:off + nplanes * W] = q_pad[gi].reshape(
                n_cores, P, nplanes * W, F)
            off += nplanes * W

    id2 = np.zeros((P, 2 * P), dtype=_NP8E4)
    k = np.arange(P)
    id2[k, k] = 1.0
    id2[k, P + k] = 1.0

    in_maps = [{"msg": msg_dev[c], "ident": id2} for c in range(n_cores)]
    return in_maps, groups, node_tbl


def kernel_v5(msg: np.ndarray, edge_index: np.ndarray) -> np.ndarray:
    msg = np.asarray(msg)
    edge_index = np.asarray(edge_index)

    in_maps, groups, node_tbl = prepare_inputs_v5(msg, edge_index)
    nc = build_program_v5(groups, N_CORES)
    res = run_bass_kernel_spmd(nc, in_maps, list(range(N_CORES)))
    out = np.zeros((NUM_NODES, F), dtype=np.float32)
    for k in range(N_CORES):
        o = res.results[k]["out"].astype(np.float32).reshape(P, NPOS, F)
        nodes = node_tbl[:, k, :]                          # [i, p]
        valid = nodes >= 0
        out[nodes[valid]] = o.transpose(1, 0, 2)[valid]
    return out


def kernel(msg: np.ndarray, edge_index: np.ndarray) -> np.ndarray:
    return kernel_v5(msg, edge_index)


# revision 13
# speedup vs baseline: 1.4227x; 1.1927x over previous
"""Trainium2 Bass kernel for segment_sum (GAT reduce-sum stage).

out[n, :] = sum over edges e with dst[e] == n of msg[e, :],  n in [0, 50000).

Strategy v3 (8 NeuronCores, SPMD single program): identity-stationary slot
accumulation.

  - Host sorts nodes by degree and packs 128 consecutive (sorted) nodes per
    tile; a "position" is 8 consecutive tiles, one per core, so all cores run
    an identical schedule s[i] = max degree within position i. Each node's
    edges fill slots w = 0..deg-1 of its partition row (zero-padded to s[i]).
    Because degrees are dense small integers, sorting makes the padding ~1%
    (vs ~24% for the per-node class scheme this replaces).
  - msg ships as fp8 e3m4 quantized with per-node error feedback (the
    quantization residual of slot w is carried into slot w+1), so the
    on-device fp32 sum of the quantized slots tracks the exact per-node sum;
    measured rel err ~4e-3 vs the 2e-2 budget. DMA traffic is ~10 MB/core.
  - Device: the PE stationary operand is a [128, 128] identity, loaded from
    the first matmul and identical for every matmul in the program. Each
    matmul accumulates one [128 nodes, 64 feat] slot chunk into PSUM:
    out[m, n] += sum_p I[p, m] * chunk[p, n] = chunk[m, n]. The moving side
    streams 64 columns (~27 ns/chunk); there is no per-tile weight churn,
    no on-device one-hot build, and DVE stays idle.
  - Output tiles accumulate into a persistent SBUF strip; one large store
    per pass. No collectives: cores own disjoint node sets; host scatters.
"""

import numpy as np
import ml_dtypes

import concourse.tile as tile
from concourse import bass, mybir
from concourse.bass_utils import run_bass_kernel_spmd
from concourse.vector_clock import ScopedClock

P = 128          # partitions / tile node count
F = 64           # feature dim
N_CORES = 8
NUM_NODES = 50000
NPOS = -(-NUM_NODES // (P * N_CORES))          # 49 positions per core
NODE_SLOTS = NPOS * P * N_CORES                # 50176 (176 dummy slots)

_FP8 = mybir.dt.float8e3                       # e3m4
_NP8 = ml_dtypes.float8_e3m4
_FP32 = mybir.dt.float32

_DMA_BATCH_CHUNKS = 256    # 256 * 64 B = 16 KiB / partition per load (~2 MB)

_MAX_INST_WAITS = 1


def _split_excess_waits(nc, max_waits: int = _MAX_INST_WAITS):
    """This walrus build rejects instructions carrying more than `max_waits`
    sem waits ("Too many sync wait commands"), but Tile's wait pass piles
    every needed proc wait onto the consuming instruction. Hoist the excess
    onto wait-only EventSemaphore instructions inserted just before, on the
    same engine queue (same semantics: queue is in-order)."""
    n = 0
    for f in nc.m.functions:
        for b in f.blocks:
            il = b.instructions
            out = []
            changed = False
            for inst in il:
                si = inst.sync_info
                if si is not None and si.on_wait and len(si.on_wait) > max_waits:
                    waits = list(si.on_wait)
                    extra, keep = waits[:-max_waits], waits[-max_waits:]
                    for i in range(0, len(extra), max_waits):
                        ev = mybir.InstEventSemaphore(
                            name=f"{inst.name}-wsplit{n}",
                            engine=inst.engine,
                            ins=[],
                            outs=[],
                            sync_info=mybir.SyncInfo(
                                on_wait=extra[i:i + max_waits], on_update=[]),
                        )
                        n += 1
                        out.append(ev)
                    inst.sync_info = mybir.SyncInfo(
                        on_wait=keep, on_update=list(si.on_update))
                    changed = True
                out.append(inst)
            if changed:
                b.instructions = out


def _patched_drain_and_barrier(self, tick_clock, wait_clock):
    nc = self.nc
    probe = nc.sync.nop(nofuse=True, hint="drain_waits")
    wait_clock.add_sem_waits(probe.ins, ScopedClock({None: tick_clock.global_clock}))
    si = probe.ins.sync_info
    waits = list(si.on_wait) if si is not None else []
    if si is not None:
        del si.on_wait[:]
    by_name = {h.name: h for h in self.sems.allocated().values()}
    for w in waits:
        assert w.wait_reg is None
        nc.sync.wait_ge(by_name[w.ant_name], w.wait_value)
    nc.sync.drain()

    nc.all_engine_barrier()
    popped = nc._tile_sem_poison_stack.pop()
    assert popped is self._sem_poison
    nc.clear_and_free_semaphores(list(self.sems.allocated().values()))
    nc.all_engine_barrier()

    _split_excess_waits(nc)


tile.TileContext._drain_and_barrier = _patched_drain_and_barrier


def build_program_v3(schedule, n_cores: int = N_CORES,
                     repeat: int = 1) -> bass.Bass:
    """schedule: per-position slot counts s[i] (same for every core).

    Inputs per core: msg [P, TOT*F] fp8 (TOT = sum(s); position i occupies
    chunk columns off[i]..off[i]+s[i], chunk w = slot-w edge of the
    partition's node), ident [P, P] fp8. Output: [P, NPOS*F] fp32
    (position-major strip; host transposes/scatters).

    repeat > 1 re-runs the whole body (for steady-state timing via the
    T(N) slope; each repeat writes the same output)."""
    npos = len(schedule)
    tot = int(sum(schedule))
    nc = bass.Bass("TRN2", target_bir_lowering=False, debug=False,
                   num_devices=n_cores)
    msg_d = nc.dram_tensor("msg", [P, tot * F], _FP8,
                           kind="ExternalInput").ap()
    id_d = nc.dram_tensor("ident", [P, P], _FP8, kind="ExternalInput").ap()
    out_d = nc.dram_tensor("out", [P, npos * F], _FP32,
                           kind="ExternalOutput").ap()

    # batch positions into DMA loads of <= _DMA_BATCH_CHUNKS chunks
    batches = []          # (chunk_off, nchunks, [(pos, local_chunk_off), ...])
    cur_off, cur_n, cur_pos = 0, 0, []
    off = 0
    for i, s in enumerate(schedule):
        if cur_n + s > _DMA_BATCH_CHUNKS and cur_n > 0:
            batches.append((cur_off, cur_n, cur_pos))
            cur_off, cur_n, cur_pos = off, 0, []
        cur_pos.append((i, cur_n))
        cur_n += s
        off += s
    if cur_n:
        batches.append((cur_off, cur_n, cur_pos))

    with tile.TileContext(nc) as tc:
        with (
            tc.tile_pool(name="const", bufs=1) as cpool,
            tc.tile_pool(name="msg", bufs=3) as mpool,
            tc.tile_pool(name="psum", bufs=8, space="PSUM") as ppool,
        ):
            id_t = cpool.tile([P, P], _FP8)
            nc.sync.dma_start(out=id_t[:], in_=id_d[:])
            out_sb = cpool.tile([P, npos * F], _FP32)

            for _r in range(repeat):
                for chunk_off, nchunks, poss in batches:
                    mt = mpool.tile([P, _DMA_BATCH_CHUNKS * F], _FP8)
                    nc.sync.dma_start(
                        out=mt[:, :nchunks * F],
                        in_=msg_d[:, chunk_off * F:(chunk_off + nchunks) * F])
                    for i, loc in poss:
                        s = schedule[i]
                        ps = ppool.tile([P, F], _FP32)
                        for w in range(s):
                            c = (loc + w) * F
                            nc.tensor.matmul(
                                out=ps[:],
                                lhsT=id_t[:],
                                rhs=mt[:, c:c + F],
                                start=(w == 0),
                                stop=(w == s - 1),
                            )
                        nc.scalar.copy(out=out_sb[:, i * F:(i + 1) * F],
                                       in_=ps[:])
                nc.gpsimd.dma_start(out=out_d[:], in_=out_sb[:])
    return nc


def _quantize_error_feedback(msg, order, starts, deg, np8=_NP8):
    """Per-node error-feedback fp8 quantization: q[e] = fp8(x[e] + carry),
    carry = running residual within the node's edge list, so the node sum of
    q tracks the exact sum to ~one final-carry. Returns q [E, F] fp8."""
    E = msg.shape[0]
    q = np.empty((E, F), dtype=np8)
    m = msg[order].astype(np.float32)
    n_nodes = len(deg)
    carry = np.zeros((n_nodes, F), np.float32)
    maxdeg = int(deg.max())
    base = starts[:-1]
    for w in range(maxdeg):
        sel = deg > w
        rows = base[sel] + w
        x = m[rows] + carry[sel]
        qx = x.astype(np8)
        q[order[rows]] = qx
        carry[sel] = x - qx.astype(np.float32)
    return q


def prepare_inputs_v3(msg: np.ndarray, edge_index: np.ndarray,
                      num_nodes: int = NUM_NODES, n_cores: int = N_CORES):
    """Returns (in_maps, schedule, node_tbl) where node_tbl[i, k, p] is the
    global node id owning output row (partition p, position i) on core k
    (-1 = dummy padding slot)."""
    E, feat = msg.shape
    assert feat == F

    dst = np.asarray(edge_index[1]).astype(np.int64)
    deg = np.bincount(dst, minlength=num_nodes)
    order = np.argsort(dst, kind="stable")         # edges grouped by dst
    starts = np.zeros(num_nodes + 1, dtype=np.int64)
    np.cumsum(deg, out=starts[1:])

    q = _quantize_error_feedback(np.asarray(msg), order, starts, deg)
    q_pad = np.concatenate([q, np.zeros((1, F), _NP8)], axis=0)

    # edge id per (node, slot): eid[n, w] = w-th edge of node n (E = none)
    maxdeg = int(deg.max())
    eid = np.full((num_nodes + 1, maxdeg), E, dtype=np.int64)
    pos_in_node = np.arange(E) - starts[dst[order]]
    eid[dst[order], pos_in_node] = order

    # degree-sorted node table: 176 dummy slots first, then nodes ascending
    node_order = np.argsort(deg, kind="stable")
    node_tbl = np.full(NODE_SLOTS, -1, dtype=np.int64)
    node_tbl[NODE_SLOTS - num_nodes:] = node_order
    node_tbl = node_tbl.reshape(NPOS, n_cores, P)

    deg_pad = np.concatenate([deg, [0]])
    schedule = deg_pad[node_tbl].max(axis=(1, 2)).astype(np.int64)
    assert schedule.min() >= 1
    tot = int(schedule.sum())
    offs = np.zeros(NPOS + 1, dtype=np.int64)
    np.cumsum(schedule, out=offs[1:])

    msg_dev = np.zeros((n_cores, P, tot * F), dtype=_NP8)
    md = msg_dev.reshape(n_cores, P, tot, F)
    for i in range(NPOS):
        s = int(schedule[i])
        g = eid[node_tbl[i], :s]                   # [n_cores, P, s]
        md[:, :, offs[i]:offs[i] + s] = q_pad[g]

    ident = np.zeros((P, P), dtype=_NP8)
    np.fill_diagonal(ident, 1.0)

    in_maps = [{"msg": msg_dev[k], "ident": ident} for k in range(n_cores)]
    return in_maps, schedule, node_tbl


def kernel_v3(msg: np.ndarray, edge_index: np.ndarray) -> np.ndarray:
    msg = np.asarray(msg)
    edge_index = np.asarray(edge_index)

    in_maps, schedule, node_tbl = prepare_inputs_v3(msg, edge_index)
    nc = build_program_v3(schedule, N_CORES)
    res = run_bass_kernel_spmd(nc, in_maps, list(range(N_CORES)))
    out = np.zeros((NUM_NODES, F), dtype=np.float32)
    for k in range(N_CORES):
        o = res.results[k]["out"].reshape(P, NPOS, F)      # [p, i, f]
        nodes = node_tbl[:, k, :]                          # [i, p]
        valid = nodes >= 0
        out[nodes[valid]] = o.transpose(1, 0, 2)[valid]
    return out


# ---------------------------------------------------------------------------
# V4: grouped slot-major layout + fp8e4 DoubleRow.
#
# v3 measured ~55 ns/chunk: each self-loading matmul re-streams the 128-col
# identity through the weight port, and at N=64 that load is the critical
# path. v4 groups 8 positions per matmul (slot-major interleave, N=512
# moving columns) so one weight load covers 8 chunks, and uses fp8e4
# DoubleRow (two slot chunks packed per partition row) to stream 2 chunks
# per position per matmul. At N=512 the moving stream (~240 ns) exceeds
# even an un-elided 256-col DoubleRow weight load (~213 ns), so the weight
# path is hidden either way. The degree-sorted position order keeps the
# per-group slot cap S_g = max(s[i]) tight (~4% padding).
# ---------------------------------------------------------------------------

_NP8E4 = ml_dtypes.float8_e4m3
_FP8E4 = mybir.dt.float8e4
_GROUP = 7     # 49 positions = 7 groups of 7; N = 7*64 = 448 <= 512 psum bank


def _v4_groups(schedule):
    """[(pos0, G, S)] covering positions; S rounded even for DoubleRow."""
    groups = []
    for pos0 in range(0, len(schedule), _GROUP):
        G = min(_GROUP, len(schedule) - pos0)
        S = int(max(schedule[pos0:pos0 + G]))
        if G * F >= 256:          # DoubleRow groups need even slot count
            S += S % 2
        groups.append((pos0, G, S))
    return groups


def build_program_v4(groups, n_cores: int = N_CORES,
                     repeat: int = 1) -> bass.Bass:
    """groups: [(pos0, G, S)]; msg is [P, sum(G*S)*F] fp8e4 slot-major
    within each group: col block (g, w, j) = slot-w chunk of position
    pos0+j. id2 is the DoubleRow identity [P, 2*P]; out [P, npos*F] f32."""
    npos = max(pos0 + G for pos0, G, _ in groups)
    tot = sum(G * S for _, G, S in groups)
    nc = bass.Bass("TRN2", target_bir_lowering=False, debug=False,
                   num_devices=n_cores)
    msg_d = nc.dram_tensor("msg", [P, tot * F], _FP8E4,
                           kind="ExternalInput").ap()
    id_d = nc.dram_tensor("ident", [P, 2 * P], _FP8E4,
                          kind="ExternalInput").ap()
    out_d = nc.dram_tensor("out", [P, npos * F], _FP32,
                           kind="ExternalOutput").ap()

    max_cols = max(G * S for _, G, S in groups) * F

    with tile.TileContext(nc) as tc:
        with (
            tc.tile_pool(name="const", bufs=1) as cpool,
            tc.tile_pool(name="msg", bufs=3) as mpool,
            tc.tile_pool(name="psum", bufs=4, space="PSUM") as ppool,
        ):
            id2_t = cpool.tile([P, 2 * P], _FP8E4)
            nc.sync.dma_start(out=id2_t[:], in_=id_d[:])
            out_sb = cpool.tile([P, npos * F], _FP32)

            id2_3d = id2_t[:].rearrange("p (t m) -> p t m", t=2)
            for _r in range(repeat):
                off = 0
                for pos0, G, S in groups:
                    ncols = G * S * F
                    mt = mpool.tile([P, max_cols], _FP8E4)
                    nc.sync.dma_start(out=mt[:, :ncols],
                                      in_=msg_d[:, off * F:off * F + ncols])
                    off += G * S
                    if G * F >= 256:
                        ps = ppool.tile([P, G * F], _FP32, tag="psdr")
                        w2 = 2 * G * F
                        for ww in range(S // 2):
                            nc.tensor.matmul(
                                out=ps[:],
                                lhsT=id2_3d,
                                rhs=mt[:, ww * w2:(ww + 1) * w2]
                                    .rearrange("p (t n) -> p t n", t=2),
                                start=(ww == 0),
                                stop=(ww == S // 2 - 1),
                                perf_mode=mybir.MatmulPerfMode.DoubleRow,
                            )
                    else:                    # tail group: single-row
                        ps = ppool.tile([P, G * F], _FP32, tag="pssr")
                        for w in range(S):
                            nc.tensor.matmul(
                                out=ps[:],
                                lhsT=id2_t[:, P:2 * P],   # plain identity
                                rhs=mt[:, w * G * F:(w + 1) * G * F],
                                start=(w == 0),
                                stop=(w == S - 1),
                            )
                    nc.scalar.copy(
                        out=out_sb[:, pos0 * F:(pos0 + G) * F], in_=ps[:])
                nc.gpsimd.dma_start(out=out_d[:], in_=out_sb[:])
    return nc


def prepare_inputs_v4(msg: np.ndarray, edge_index: np.ndarray,
                      num_nodes: int = NUM_NODES, n_cores: int = N_CORES):
    """Returns (in_maps, groups, node_tbl)."""
    E, feat = msg.shape
    assert feat == F

    dst = np.asarray(edge_index[1]).astype(np.int64)
    deg = np.bincount(dst, minlength=num_nodes)
    order = np.argsort(dst, kind="stable")
    starts = np.zeros(num_nodes + 1, dtype=np.int64)
    np.cumsum(deg, out=starts[1:])

    q = _quantize_error_feedback(np.asarray(msg), order, starts, deg,
                                 np8=_NP8E4)
    q_pad = np.concatenate([q, np.zeros((1, F), _NP8E4)], axis=0)

    maxdeg = int(deg.max())
    eid = np.full((num_nodes + 1, maxdeg), E, dtype=np.int64)
    pos_in_node = np.arange(E) - starts[dst[order]]
    eid[dst[order], pos_in_node] = order

    node_order = np.argsort(deg, kind="stable")
    node_tbl = np.full(NODE_SLOTS, -1, dtype=np.int64)
    node_tbl[NODE_SLOTS - num_nodes:] = node_order
    node_tbl = node_tbl.reshape(NPOS, n_cores, P)

    deg_pad = np.concatenate([deg, [0]])
    schedule = deg_pad[node_tbl].max(axis=(1, 2)).astype(np.int64)
    groups = _v4_groups(schedule)
    tot = sum(G * S for _, G, S in groups)

    msg_dev = np.zeros((n_cores, P, tot * F), dtype=_NP8E4)
    md = msg_dev.reshape(n_cores, P, tot, F)
    off = 0
    for pos0, G, S in groups:
        nodes_g = node_tbl[pos0:pos0 + G]            # [G, cores, P]
        gi = eid[nodes_g, :S]                        # [G, cores, P, S]
        # -> [cores, P, S, G]
        gi = gi.transpose(1, 2, 3, 0)
        md[:, :, off:off + S * G] = q_pad[gi].reshape(
            n_cores, P, S * G, F)
        off += S * G

    id2 = np.zeros((P, 2 * P), dtype=_NP8E4)
    k = np.arange(P)
    id2[k, k] = 1.0            # t = 0 plane
    id2[k, P + k] = 1.0        # t = 1 plane (also the plain identity slice)

    in_maps = [{"msg": msg_dev[c], "ident": id2} for c in range(n_cores)]
    return in_maps, groups, node_tbl


def kernel_v4(msg: np.ndarray, edge_index: np.ndarray) -> np.ndarray:
    msg = np.asarray(msg)
    edge_index = np.asarray(edge_index)

    in_maps, groups, node_tbl = prepare_inputs_v4(msg, edge_index)
    nc = build_program_v4(groups, N_CORES)
    res = run_bass_kernel_spmd(nc, in_maps, list(range(N_CORES)))
    out = np.zeros((NUM_NODES, F), dtype=np.float32)
    for k in range(N_CORES):
        o = res.results[k]["out"].reshape(P, NPOS, F)      # [p, i, f]
        nodes = node_tbl[:, k, :]                          # [i, p]
        valid = nodes >= 0
        out[nodes[valid]] = o.transpose(1, 0, 2)[valid]
    return out


# ---------------------------------------------------------------------------
# V5: ragged slot compaction + bf16 output.
#
# v4 pads every position of a group to the group-max slot count (+11% DMA).
# v5 orders each group's positions by descending slot count, so slot slab
# w only carries the prefix of positions still active (width W_w). Each
# DoubleRow matmul pair accumulates into a shrinking PREFIX of the psum
# tile (per-element has_written semantics make partial accumulation safe;
# pair 0 has full width and carries start=True). Output is stored as bf16
# (host upcasts), halving the write traffic; the added rounding (~2e-3 of
# scale) is well inside the error budget.
# ---------------------------------------------------------------------------

_BF16 = mybir.dt.bfloat16
_NPBF16 = ml_dtypes.bfloat16


def build_program_v5(groups, n_cores: int = N_CORES,
                     repeat: int = 1) -> bass.Bass:
    """groups: [(pos0, G, widths)]; for a DoubleRow group (G*F >= 256),
    widths[ww] = position-prefix width of slot pair (2ww, 2ww+1) and the
    msg slab for the pair is [2 planes x widths[ww]*F] columns; for a
    single-row group widths[w] = width of slot w (one plane)."""
    npos = max(pos0 + G for pos0, G, _ in groups)

    def group_chunks(G, widths):
        return sum((2 if G * F >= 256 else 1) * W for W in widths)

    tot = sum(group_chunks(G, ws) for _, G, ws in groups)
    nc = bass.Bass("TRN2", target_bir_lowering=False, debug=False,
                   num_devices=n_cores)
    msg_d = nc.dram_tensor("msg", [P, tot * F], _FP8E4,
                           kind="ExternalInput").ap()
    id_d = nc.dram_tensor("ident", [P, 2 * P], _FP8E4,
                          kind="ExternalInput").ap()
    out_d = nc.dram_tensor("out", [P, npos * F], _BF16,
                           kind="ExternalOutput").ap()

    max_cols = max(group_chunks(G, ws) for _, G, ws in groups) * F

    with tile.TileContext(nc) as tc:
        with (
            tc.tile_pool(name="const", bufs=1) as cpool,
            tc.tile_pool(name="msg", bufs=4) as mpool,
            tc.tile_pool(name="outp", bufs=2) as opool,
            tc.tile_pool(name="psum", bufs=4, space="PSUM") as ppool,
        ):
            id2_t = cpool.tile([P, 2 * P], _FP8E4)
            nc.sync.dma_start(out=id2_t[:], in_=id_d[:])

            id2_3d = id2_t[:].rearrange("p (t m) -> p t m", t=2)
            if repeat == 1:
                # single-shot: ~3.4us of junk matmuls while the first msg
                # load streams, so the PE HAM clock-gate is released (1.2 ->
                # 2.4 GHz) before the real accumulation chains start
                wps = ppool.tile([P, _GROUP * F], _FP32, tag=f"ps{_GROUP}")
                for _ in range(32):
                    nc.tensor.matmul(out=wps[:, :P], lhsT=id2_t[:, P:2 * P],
                                     rhs=id2_t[:, :P], start=True, stop=True)
            dma_rings = [nc.sync, nc.sync]
            for _r in range(repeat):
                out_sb = opool.tile([P, npos * F], _BF16)
                off = 0
                for gi_, (pos0, G, widths) in enumerate(groups):
                    ncols = group_chunks(G, widths) * F
                    mt = mpool.tile([P, max_cols], _FP8E4)
                    dma_rings[gi_ % 2].dma_start(
                        out=mt[:, :ncols],
                        in_=msg_d[:, off * F:off * F + ncols])
                    off += group_chunks(G, widths)
                    ps = ppool.tile([P, G * F], _FP32, tag=f"ps{G}")
                    c = 0
                    if G * F >= 256:
                        for ww, W in enumerate(widths):
                            nc.tensor.matmul(
                                out=ps[:, :W * F],
                                lhsT=id2_3d,
                                rhs=mt[:, c:c + 2 * W * F]
                                    .rearrange("p (t n) -> p t n", t=2),
                                start=(ww == 0),
                                stop=(ww == len(widths) - 1),
                                perf_mode=mybir.MatmulPerfMode.DoubleRow,
                                skip_group_check=True,
                            )
                            c += 2 * W * F
                    else:
                        for w, W in enumerate(widths):
                            nc.tensor.matmul(
                                out=ps[:, :W * F],
                                lhsT=id2_t[:, P:P + P],
                                rhs=mt[:, c:c + W * F],
                                start=(w == 0),
                                stop=(w == len(widths) - 1),
                                skip_group_check=True,
                            )
                            c += W * F
                    nc.scalar.copy(
                        out=out_sb[:, pos0 * F:(pos0 + G) * F], in_=ps[:])
                nc.gpsimd.dma_start(out=out_d[:], in_=out_sb[:])
    return nc


def prepare_inputs_v5(msg: np.ndarray, edge_index: np.ndarray,
                      num_nodes: int = NUM_NODES, n_cores: int = N_CORES):
    """Returns (in_maps, groups, node_tbl); groups as build_program_v5."""
    E, feat = msg.shape
    assert feat == F

    dst = np.asarray(edge_index[1]).astype(np.int64)
    deg = np.bincount(dst, minlength=num_nodes)
    order = np.argsort(dst, kind="stable")
    starts = np.zeros(num_nodes + 1, dtype=np.int64)
    np.cumsum(deg, out=starts[1:])

    q = _quantize_error_feedback(np.asarray(msg), order, starts, deg,
                                 np8=_NP8E4)
    q_pad = np.concatenate([q, np.zeros((1, F), _NP8E4)], axis=0)

    maxdeg = int(deg.max())
    # one spare all-empty slot column so odd pair planes can index slot S
    eid = np.full((num_nodes + 1, maxdeg + 1), E, dtype=np.int64)
    pos_in_node = np.arange(E) - starts[dst[order]]
    eid[dst[order], pos_in_node] = order

    node_order = np.argsort(deg, kind="stable")
    node_tbl = np.full(NODE_SLOTS, -1, dtype=np.int64)
    node_tbl[NODE_SLOTS - num_nodes:] = node_order
    node_tbl = node_tbl.reshape(NPOS, n_cores, P)

    deg_pad = np.concatenate([deg, [0]])
    schedule = deg_pad[node_tbl].max(axis=(1, 2)).astype(np.int64)

    # within each group of _GROUP positions, order positions by descending
    # slot count so slot slabs are prefix-shaped
    groups = []
    for pos0 in range(0, NPOS, _GROUP):
        G = min(_GROUP, NPOS - pos0)
        perm = np.argsort(-schedule[pos0:pos0 + G], kind="stable")
        node_tbl[pos0:pos0 + G] = node_tbl[pos0 + perm]
        schedule[pos0:pos0 + G] = schedule[pos0 + perm]
        s = schedule[pos0:pos0 + G]
        S = int(s[0])
        if G * F >= 256:
            widths = [int((s >= 2 * ww + 1).sum())
                      for ww in range((S + 1) // 2)]
        else:
            widths = [int((s >= w + 1).sum()) for w in range(S)]
        groups.append((pos0, G, widths))

    def group_chunks(G, widths):
        return sum((2 if G * F >= 256 else 1) * W for W in widths)

    tot = sum(group_chunks(G, ws) for _, G, ws in groups)

    msg_dev = np.zeros((n_cores, P, tot * F), dtype=_NP8E4)
    md = msg_dev.reshape(n_cores, P, tot, F)
    off = 0
    for pos0, G, widths in groups:
        nplanes = 2 if G * F >= 256 else 1
        for ww, W in enumerate(widths):
            nodes_w = node_tbl[pos0:pos0 + W]            # [W, cores, P]
            slots = [nplanes * ww + t for t in range(nplanes)]
            gi = eid[nodes_w][:, :, :, slots]            # [W, cores, P, t]
            gi = gi.transpose(1, 2, 3, 0)                # [cores, P, t, W]
            md[:, :, off:off + nplanes * W] = q_pad[gi].reshape(
                n_cores, P, nplanes * W, F)
            off += nplanes * W

    id2 = np.zeros((P, 2 * P), dtype=_NP8E4)
    k = np.arange(P)
    id2[k, k] = 1.0
    id2[k, P + k] = 1.0

    in_maps = [{"msg": msg_dev[c], "ident": id2} for c in range(n_cores)]
    return in_maps, groups, node_tbl


def kernel_v5(msg: np.ndarray, edge_index: np.ndarray) -> np.ndarray:
    msg = np.asarray(msg)
    edge_index = np.asarray(edge_index)

    in_maps, groups, node_tbl = prepare_inputs_v5(msg, edge_index)
    nc = build_program_v5(groups, N_CORES)
    res = run_bass_kernel_spmd(nc, in_maps, list(range(N_CORES)))
    out = np.zeros((NUM_NODES, F), dtype=np.float32)
    for k in range(N_CORES):
        o = res.results[k]["out"].astype(np.float32).reshape(P, NPOS, F)
        nodes = node_tbl[:, k, :]                          # [i, p]
        valid = nodes >= 0
        out[nodes[valid]] = o.transpose(1, 0, 2)[valid]
    return out


def kernel(msg: np.ndarray, edge_index: np.ndarray) -> np.ndarray:
    return kernel_v5(msg, edge_index)
